# revision 24
# baseline (speedup 1.0000x reference)
"""LoRA layer kernel for Trainium2 (8 NeuronCores, data-parallel over rows).

Computes out = ((x @ V^T) * S) @ U^T * scaling  (scaling = alpha/rank = 1.0)
for x [4, 2048, 4096], U [4096, 32], S [32], V [32, 4096], all fp32.

Sharding: batch*seq rows (8192) split evenly across the 8 cores; the tiny
LoRA factors are replicated. All layout prep happens on the host:
  - x is cast to bf16 and pre-transposed/tiled to [chunk, p, ft, row] so the
    device reads features-on-partitions directly (no on-device transposes)
  - V is cast to bf16 and pre-tiled to [p, ft, 4*rank]: the 32 V rows are
    stacked 4x in mm1's stationary operand, so mm1 emits hT replicated into
    all four 32-partition row groups at no extra PE cost (matmul time
    scales with the moving free dim, not the stationary width)
  - U is scaled by S*scaling, transposed, cast to bf16, and replicated 4x
    across partitions (usT4[p] = usT[p % 32])
Output is written bf16 (halves the store traffic) and upcast to fp32 on the
host; bf16 keeps max rel err ~5e-3 against the fp32 reference.

Per core (1024 rows, 4 chunks of 256):
  - all DMAs ride the SP (sync) HWDGE ring; the FIFO gives input loads
    strict priority over output stores (the back half is PSUM-evacuation
    limited anyway, and early input completion unblocks the next mm1)
  - mm1: hT4[128, 256] += vsT4[:, ft, :]^T @ xt[:, ft, :] accumulated over
    32 feature tiles in one PSUM bank; 1 MiB half-chunk DMA granularity
  - mm2: 16 matmuls [128, 512] per chunk, packed 4-at-a-time into the four
    32-row PE tiles via tile_position=(32g, 0) (K=32 contraction), so mm2
    runs ~4x concurrent and the PE never gates PSUM evacuation even when
    the HAM clock gate has it throttled to 1.2 GHz
  - mm2 of chunk c-1 alternates with mm1 of chunk c; PSUM->SBUF copies
    split DVE/ScalarE 50/50 with bf16 downcast; per-row-tile 1 MiB stores
  - a short burst of dummy matmuls on zeroed scratch bridges the PE from
    engine boot to the first x arrival so the HAM lifts 1.2->2.4 GHz early
Roofline: ~18 MiB HBM traffic per core at ~360-425 GB/s => ~45-50 us;
PE ~20 us and copies ~22 us hidden under DMA. No collectives needed.
"""

import sys

for _p in ("/root/.axon_site/_ro/trn_rl_repo", "/opt/trn_rl_repo"):
    if _p not in sys.path:
        sys.path.append(_p)

import ml_dtypes
import numpy as np

import concourse.bass as bass
from concourse import mybir
from concourse.bass_utils import run_bass_kernel_spmd
from concourse.tile import TileContext

F32 = mybir.dt.float32
BF16 = mybir.dt.bfloat16
NP_BF16 = ml_dtypes.bfloat16

P = 128
ROWS = 1024  # per-core row shard
FEAT = 4096
RANK = 32
NG = P // RANK  # 4 row groups
SCALING = 1.0  # alpha / max_rank = 32 / 32
FT = FEAT // P  # 32 feature tiles
CHUNK = 256  # rows per pipeline chunk
CHUNK_TILES = CHUNK // P  # 2
N_CHUNKS = ROWS // CHUNK  # 4
NQ = 2  # input DMA halves per chunk (1 MiB each)
FQ = FT // NQ  # 16 feature tiles per half
OC = FEAT // 512  # 8 output column chunks per row tile
N_CORES = 8


def _split_multiwaits(nc) -> None:
    # Workaround for this container's walrus: engine instructions with >=2
    # sem waits fail codegen ("Too many sync wait commands"). Hoist all but
    # the last wait onto single-wait NoOps inserted just before, same engine.
    for f in nc.m.functions:
        for bb in f.blocks:
            out = []
            changed = False
            for inst in bb.instructions:
                si = inst.sync_info
                waits = list(si.on_wait) if (si is not None and si.on_wait) else []
                if len(waits) > 1:
                    changed = True
                    for w in waits[:-1]:
                        nop = mybir.InstNoOp(name=f"splitw-{nc.next_id()}")
                        nop.engine = inst.engine
                        nop.sync_info = mybir.SyncInfo(on_wait=[w], on_update=[])
                        nc.register_instruction(nop)
                        out.append(nop)
                    si.on_wait = [waits[-1]]
                out.append(inst)
            if changed:
                bb.instructions = out


class _PatchedTileContext(TileContext):
    def _drain_and_barrier(self, tick_clock, wait_clock):
        super()._drain_and_barrier(tick_clock, wait_clock)
        _split_multiwaits(self.nc)


def build_nc() -> bass.Bass:
    nc = bass.Bass(trn_type="TRN2", target_bir_lowering=False, name="lora")
    # xt host layout: [chunk, p, ft, row-in-chunk]; half-chunk slices are
    # 1 MiB contiguous-per-partition DMAs
    xt_d = nc.dram_tensor("xt", [N_CHUNKS, P, FT * CHUNK], BF16, kind="ExternalInput")
    vt_d = nc.dram_tensor("vt", [P, FT * P], BF16, kind="ExternalInput")
    ut_d = nc.dram_tensor("ut", [P, FEAT], BF16, kind="ExternalInput")
    out_d = nc.dram_tensor("out", [ROWS, FEAT], BF16, kind="ExternalOutput")

    with _PatchedTileContext(nc) as tc:
        with (
            tc.tile_pool(name="consts", bufs=1) as consts,
            tc.tile_pool(name="xin", bufs=N_CHUNKS) as x_pool,
            tc.tile_pool(name="hts", bufs=2) as h_pool,
            tc.tile_pool(name="outs", bufs=2) as out_pool,
            tc.tile_pool(name="ps_h", bufs=2, space="PSUM") as psum_h,
            tc.tile_pool(name="ps_o", bufs=3, space="PSUM") as psum_o,
        ):
            # issue order on the sync ring = need order, finely interleaved
            # so mm1(0) can start as early as possible:
            # vt half, x0 half, vt half, x0 half, ut, x1.. halves, stores
            vsT4 = consts.tile([P, FT, P], BF16)
            vsrc = vt_d[:, :].rearrange("p (f q) -> p f q", q=P)

            x_tiles = []
            srcs = []
            for c in range(N_CHUNKS):
                xt = x_pool.tile([P, FT, CHUNK], BF16, tag="x")
                x_tiles.append(xt)
                srcs.append(xt_d[c, :, :].rearrange("p (f r) -> p f r", r=CHUNK))
            usT4 = consts.tile([P, FEAT], BF16)
            for c in range(N_CHUNKS):
                for q in range(NQ):
                    if c == 0:
                        nc.sync.dma_start(
                            vsT4[:, q * FQ : (q + 1) * FQ, :],
                            vsrc[:, q * FQ : (q + 1) * FQ, :],
                        )
                    nc.sync.dma_start(
                        x_tiles[c][:, q * FQ : (q + 1) * FQ, :],
                        srcs[c][:, q * FQ : (q + 1) * FQ, :],
                    )
                if c == 0:
                    # ut is first needed by mm2(0), after chunk 0 is loaded
                    nc.sync.dma_start(usT4, ut_d[:, :])

            # dummy matmuls on zeroed scratch bridge the PE from engine boot
            # until the first x half lands, so the HAM activity window sees
            # continuous busy-ness and lifts the clock to 2.4 GHz during
            # mm1(0) instead of halfway through the kernel. One persistent
            # scratch PSUM bank (shared slot in the ps_h pool), alternating
            # halves: consecutive dummies have no pool-release semaphores
            # (same-engine WAW = program order).
            warm_sb = consts.tile([P, 512], BF16)
            nc.vector.memset(warm_sb, 0.0)
            ps_w = psum_h.tile([P, 512], F32, tag="h")
            for w in range(12):
                nc.tensor.matmul(
                    ps_w,
                    warm_sb[:, :P],
                    warm_sb,
                    start=True,
                    stop=True,
                    skip_group_check=True,
                )

            def emit_mm1_ft(c, ps_h, ft):
                nc.tensor.matmul(
                    ps_h,
                    vsT4[:, ft, :],
                    x_tiles[c][:, ft, :],
                    start=(ft == 0),
                    stop=(ft == FT - 1),
                    skip_group_check=True,
                )

            def emit_mm2_rt(hT4, out_sb, ci, rt):
                # one row tile: 8 matmuls in two 4-packs spread across the
                # four 32-row PE tiles (K=32) — they run concurrently, so the
                # PE never gates PSUM evacuation. Each pair of matmuls fills
                # one 2-bank PSUM tile that a single [128, 1024] copy
                # evacuates (halves the copy instruction count); copies
                # alternate DVE/ACT; then the 1 MiB store.
                pss = []
                for pk in range(OC // NG):
                    for gp in range(NG // 2):
                        ps_o = psum_o.tile([P, 1024], F32, tag="po")
                        for h in range(2):
                            g = gp * 2 + h
                            oc = pk * NG + g
                            nc.tensor.matmul(
                                ps_o[:, h * 512 : (h + 1) * 512],
                                hT4[g * RANK : (g + 1) * RANK, rt * P : (rt + 1) * P],
                                usT4[
                                    g * RANK : (g + 1) * RANK,
                                    oc * 512 : (oc + 1) * 512,
                                ],
                                start=True,
                                stop=True,
                                skip_group_check=True,
                                tile_position=(g * RANK, 0),
                            )
                        pss.append((pk * NG + gp * 2, ps_o))
                for j, (oc0, ps_o) in enumerate(pss):
                    dst = out_sb[:, rt, oc0 * 512 : (oc0 + 2) * 512]
                    if j % 2 == 0:
                        nc.vector.tensor_copy(out=dst, in_=ps_o)
                    else:
                        nc.scalar.copy(out=dst, in_=ps_o)
                r0 = ci * CHUNK + rt * P
                nc.sync.dma_start(out_d[r0 : r0 + P, :], out_sb[:, rt, :])

            # software pipeline: mm2 of chunk c-1 alternates with mm1 of
            # chunk c (one rt-group per input half), so the copy engines are
            # fed from chunk 1 onward and the final drain is only one
            # chunk's worth of mm2
            pending = None  # (hT4, out_sb, ci) of previous chunk
            for c in range(N_CHUNKS):
                ps_h = psum_h.tile([P, CHUNK], F32, tag="h")
                for q in range(NQ):
                    if pending is not None:
                        emit_mm2_rt(*pending, rt=q)
                    for ft in range(q * FQ, (q + 1) * FQ):
                        emit_mm1_ft(c, ps_h, ft)
                hT4 = h_pool.tile([P, CHUNK], BF16, tag="hT")
                nc.vector.tensor_copy(out=hT4, in_=ps_h)
                out_sb = out_pool.tile([P, CHUNK_TILES, FEAT], BF16, tag="out")
                pending = (hT4, out_sb, c)
            hT_l, out_l, ci_l = pending
            for rt in range(CHUNK_TILES):
                emit_mm2_rt(hT_l, out_l, ci_l, rt)
    return nc


_NC_CACHE = None


def _get_nc():
    global _NC_CACHE
    if _NC_CACHE is None:
        _NC_CACHE = build_nc()
    return _NC_CACHE


def make_in_maps(x2, U, S, V):
    xb = np.ascontiguousarray(x2, dtype=np.float32).astype(NP_BF16)
    vb = np.ascontiguousarray(V, dtype=np.float32).astype(NP_BF16)
    # vt[p, ft, g*RANK + r] = V[r, ft*P + p]  (4 stacked replicas of V rows)
    vt1 = vb.reshape(RANK, FT, P).transpose(2, 1, 0)  # [p, ft, r]
    vt = np.ascontiguousarray(
        np.broadcast_to(vt1[:, :, None, :], (P, FT, NG, RANK))
    ).reshape(P, FT * P)
    us = np.asarray(U, dtype=np.float32) * (
        np.asarray(S, dtype=np.float32)[None, :] * SCALING
    )
    ut1 = np.ascontiguousarray(us.T).astype(NP_BF16)  # [RANK, FEAT]
    ut = np.ascontiguousarray(
        np.broadcast_to(ut1[None, :, :], (NG, RANK, FEAT))
    ).reshape(P, FEAT)
    maps = []
    for i in range(N_CORES):
        xs = xb[i * ROWS : (i + 1) * ROWS]
        # xt[c, p, ft, r] = xs[c*CHUNK + r, ft*P + p]
        xt = np.ascontiguousarray(
            xs.reshape(N_CHUNKS, CHUNK, FT, P).transpose(0, 3, 2, 1)
        ).reshape(N_CHUNKS, P, FT * CHUNK)
        maps.append({"xt": xt, "vt": vt, "ut": ut})
    return maps


def kernel(**inputs) -> np.ndarray:
    x = np.asarray(inputs["x"])
    U = inputs["U"]
    S = inputs["S"]
    V = inputs["V"]

    b, sq, feat = x.shape
    x2 = x.reshape(b * sq, feat)

    nc = _get_nc()
    in_maps = make_in_maps(x2, U, S, V)
    res = run_bass_kernel_spmd(nc, in_maps, core_ids=list(range(N_CORES)))
    out = np.concatenate([r["out"] for r in res.results], axis=0)
    return out.astype(np.float32).reshape(b, sq, feat)


# revision 26
# speedup vs baseline: 1.0619x; 1.0619x over previous
"""LoRA layer kernel for Trainium2 (8 NeuronCores, data-parallel over rows).

Computes out = ((x @ V^T) * S) @ U^T * scaling  (scaling = alpha/rank = 1.0)
for x [4, 2048, 4096], U [4096, 32], S [32], V [32, 4096], all fp32.

Sharding: batch*seq rows (8192) split evenly across the 8 cores; the tiny
LoRA factors are replicated. All layout prep happens on the host:
  - x is cast to bf16 and pre-transposed/tiled to [chunk, p, ft, row] so the
    device reads features-on-partitions directly (no on-device transposes)
  - V is cast to bf16 and pre-tiled to [p, ft, 4*rank]: the 32 V rows are
    stacked 4x in mm1's stationary operand, so mm1 emits hT replicated into
    all four 32-partition row groups at no extra PE cost (matmul time
    scales with the moving free dim, not the stationary width)
  - U is scaled by S*scaling, transposed, cast to bf16, and replicated 4x
    across partitions (usT4[p] = usT[p % 32])
Output is written bf16 (halves the store traffic) and upcast to fp32 on the
host; bf16 keeps max rel err ~5e-3 against the fp32 reference.

Per core (1024 rows, 4 chunks of 256):
  - all DMAs ride the SP (sync) HWDGE ring; the FIFO gives input loads
    strict priority over output stores (the back half is PSUM-evacuation
    limited anyway, and early input completion unblocks the next mm1)
  - mm1: hT4[128, 256] += vsT4[:, ft, :]^T @ xt[:, ft, :] accumulated over
    32 feature tiles in one PSUM bank; 1 MiB half-chunk DMA granularity
  - mm2: 16 matmuls [128, 512] per chunk, packed 4-at-a-time into the four
    32-row PE tiles via tile_position=(32g, 0) (K=32 contraction), so mm2
    runs ~4x concurrent and the PE never gates PSUM evacuation even when
    the HAM clock gate has it throttled to 1.2 GHz
  - mm2 of chunk c-1 alternates with mm1 of chunk c; PSUM->SBUF copies
    split DVE/ScalarE 50/50 with bf16 downcast; per-row-tile 1 MiB stores
  - a short burst of dummy matmuls on zeroed scratch bridges the PE from
    engine boot to the first x arrival so the HAM lifts 1.2->2.4 GHz early
Roofline: ~18 MiB HBM traffic per core at ~360-425 GB/s => ~45-50 us;
PE ~20 us and copies ~22 us hidden under DMA. No collectives needed.
"""

import sys

for _p in ("/root/.axon_site/_ro/trn_rl_repo", "/opt/trn_rl_repo"):
    if _p not in sys.path:
        sys.path.append(_p)

import ml_dtypes
import numpy as np

import concourse.bass as bass
from concourse import mybir
from concourse.bass_utils import run_bass_kernel_spmd
from concourse.tile import TileContext

F32 = mybir.dt.float32
BF16 = mybir.dt.bfloat16
NP_BF16 = ml_dtypes.bfloat16

P = 128
ROWS = 1024  # per-core row shard
FEAT = 4096
RANK = 32
NG = P // RANK  # 4 row groups
SCALING = 1.0  # alpha / max_rank = 32 / 32
FT = FEAT // P  # 32 feature tiles
CHUNK = 512  # rows per pipeline chunk
CHUNK_TILES = CHUNK // P  # 2
N_CHUNKS = ROWS // CHUNK  # 4
NQ = 4  # input DMA quarters per chunk (1 MiB each)
FQ = FT // NQ  # 8 feature tiles per quarter
OC = FEAT // 512  # 8 output column chunks per row tile
N_CORES = 8


def _split_multiwaits(nc) -> None:
    # Workaround for this container's walrus: engine instructions with >=2
    # sem waits fail codegen ("Too many sync wait commands"). Hoist all but
    # the last wait onto single-wait NoOps inserted just before, same engine.
    for f in nc.m.functions:
        for bb in f.blocks:
            out = []
            changed = False
            for inst in bb.instructions:
                si = inst.sync_info
                waits = list(si.on_wait) if (si is not None and si.on_wait) else []
                if len(waits) > 1:
                    changed = True
                    for w in waits[:-1]:
                        nop = mybir.InstNoOp(name=f"splitw-{nc.next_id()}")
                        nop.engine = inst.engine
                        nop.sync_info = mybir.SyncInfo(on_wait=[w], on_update=[])
                        nc.register_instruction(nop)
                        out.append(nop)
                    si.on_wait = [waits[-1]]
                out.append(inst)
            if changed:
                bb.instructions = out


class _PatchedTileContext(TileContext):
    def _drain_and_barrier(self, tick_clock, wait_clock):
        super()._drain_and_barrier(tick_clock, wait_clock)
        _split_multiwaits(self.nc)


def build_nc() -> bass.Bass:
    nc = bass.Bass(trn_type="TRN2", target_bir_lowering=False, name="lora")
    # xt host layout: [chunk, p, ft, row-in-chunk]; half-chunk slices are
    # 1 MiB contiguous-per-partition DMAs
    xt_d = nc.dram_tensor("xt", [N_CHUNKS, P, FT * CHUNK], BF16, kind="ExternalInput")
    vt_d = nc.dram_tensor("vt", [P, FT * P], BF16, kind="ExternalInput")
    ut_d = nc.dram_tensor("ut", [P, FEAT], BF16, kind="ExternalInput")
    out_d = nc.dram_tensor("out", [ROWS, FEAT], BF16, kind="ExternalOutput")

    with _PatchedTileContext(nc) as tc:
        with (
            tc.tile_pool(name="consts", bufs=1) as consts,
            tc.tile_pool(name="xin", bufs=N_CHUNKS) as x_pool,
            tc.tile_pool(name="hts", bufs=2) as h_pool,
            tc.tile_pool(name="outs", bufs=2) as out_pool,
            tc.tile_pool(name="ps_h", bufs=2, space="PSUM") as psum_h,
            tc.tile_pool(name="ps_o", bufs=5, space="PSUM") as psum_o,
        ):
            # issue order on the sync ring = need order, finely interleaved
            # so mm1(0) can start as early as possible:
            # vt half, x0 half, vt half, x0 half, ut, x1.. halves, stores
            vsT4 = consts.tile([P, FT, P], BF16)
            vsrc = vt_d[:, :].rearrange("p (f q) -> p f q", q=P)

            x_tiles = []
            srcs = []
            for c in range(N_CHUNKS):
                xt = x_pool.tile([P, FT, CHUNK], BF16, tag="x")
                x_tiles.append(xt)
                srcs.append(xt_d[c, :, :].rearrange("p (f r) -> p f r", r=CHUNK))
            usT4 = consts.tile([P, FEAT], BF16)
            for c in range(N_CHUNKS):
                for q in range(NQ):
                    if c == 0:
                        nc.sync.dma_start(
                            vsT4[:, q * FQ : (q + 1) * FQ, :],
                            vsrc[:, q * FQ : (q + 1) * FQ, :],
                        )
                    nc.sync.dma_start(
                        x_tiles[c][:, q * FQ : (q + 1) * FQ, :],
                        srcs[c][:, q * FQ : (q + 1) * FQ, :],
                    )
                if c == 0:
                    # ut is first needed by mm2(0), after chunk 0 is loaded
                    nc.sync.dma_start(usT4, ut_d[:, :])

            # dummy matmuls on zeroed scratch bridge the PE from engine boot
            # until the first x half lands, so the HAM activity window sees
            # continuous busy-ness and lifts the clock to 2.4 GHz during
            # mm1(0) instead of halfway through the kernel. One persistent
            # scratch PSUM bank (shared slot in the ps_h pool), alternating
            # halves: consecutive dummies have no pool-release semaphores
            # (same-engine WAW = program order).
            warm_sb = consts.tile([P, 512], BF16)
            nc.vector.memset(warm_sb, 0.0)
            ps_w = psum_h.tile([P, 512], F32, tag="h")
            for w in range(12):
                nc.tensor.matmul(
                    ps_w,
                    warm_sb[:, :P],
                    warm_sb,
                    start=True,
                    stop=True,
                    skip_group_check=True,
                )

            def emit_mm1_ft(c, ps_h, ft):
                nc.tensor.matmul(
                    ps_h,
                    vsT4[:, ft, :],
                    x_tiles[c][:, ft, :],
                    start=(ft == 0),
                    stop=(ft == FT - 1),
                    skip_group_check=True,
                )

            def emit_mm2_rt(hT4, out_sb, ci, rt):
                # one row tile: 8 matmuls in two 4-packs spread across the
                # four 32-row PE tiles (K=32) — they run concurrently, so the
                # PE never gates PSUM evacuation. Each pair of matmuls fills
                # one 2-bank PSUM tile that a single [128, 1024] copy
                # evacuates (halves the copy instruction count); copies
                # alternate DVE/ACT; then the 1 MiB store.
                pss = []
                for pk in range(OC // NG):
                    for g in range(NG):
                        oc = pk * NG + g
                        ps_o = psum_o.tile([P, 512], F32, tag="po")
                        nc.tensor.matmul(
                            ps_o,
                            hT4[g * RANK : (g + 1) * RANK, rt * P : (rt + 1) * P],
                            usT4[g * RANK : (g + 1) * RANK, oc * 512 : (oc + 1) * 512],
                            start=True,
                            stop=True,
                            skip_group_check=True,
                            tile_position=(g * RANK, 0),
                        )
                        pss.append((oc, ps_o))
                for j, (oc, ps_o) in enumerate(pss):
                    dst = out_sb[:, rt, oc * 512 : (oc + 1) * 512]
                    if j % 2 == 0:
                        nc.vector.tensor_copy(out=dst, in_=ps_o)
                    else:
                        nc.scalar.copy(out=dst, in_=ps_o)
                r0 = ci * CHUNK + rt * P
                nc.sync.dma_start(out_d[r0 : r0 + P, :], out_sb[:, rt, :])

            # software pipeline: mm2 of chunk c-1 alternates with mm1 of
            # chunk c (one rt-group per input half), so the copy engines are
            # fed from chunk 1 onward and the final drain is only one
            # chunk's worth of mm2
            pending = None  # (hT4, out_sb, ci) of previous chunk
            for c in range(N_CHUNKS):
                ps_h = psum_h.tile([P, CHUNK], F32, tag="h")
                for q in range(NQ):
                    if pending is not None:
                        emit_mm2_rt(*pending, rt=q)
                    for ft in range(q * FQ, (q + 1) * FQ):
                        emit_mm1_ft(c, ps_h, ft)
                hT4 = h_pool.tile([P, CHUNK], BF16, tag="hT")
                nc.vector.tensor_copy(out=hT4, in_=ps_h)
                out_sb = out_pool.tile([P, CHUNK_TILES, FEAT], BF16, tag="out")
                pending = (hT4, out_sb, c)
            hT_l, out_l, ci_l = pending
            for rt in range(CHUNK_TILES):
                emit_mm2_rt(hT_l, out_l, ci_l, rt)
    return nc


_NC_CACHE = None


def _get_nc():
    global _NC_CACHE
    if _NC_CACHE is None:
        _NC_CACHE = build_nc()
    return _NC_CACHE


def make_in_maps(x2, U, S, V):
    xb = np.ascontiguousarray(x2, dtype=np.float32).astype(NP_BF16)
    vb = np.ascontiguousarray(V, dtype=np.float32).astype(NP_BF16)
    # vt[p, ft, g*RANK + r] = V[r, ft*P + p]  (4 stacked replicas of V rows)
    vt1 = vb.reshape(RANK, FT, P).transpose(2, 1, 0)  # [p, ft, r]
    vt = np.ascontiguousarray(
        np.broadcast_to(vt1[:, :, None, :], (P, FT, NG, RANK))
    ).reshape(P, FT * P)
    us = np.asarray(U, dtype=np.float32) * (
        np.asarray(S, dtype=np.float32)[None, :] * SCALING
    )
    ut1 = np.ascontiguousarray(us.T).astype(NP_BF16)  # [RANK, FEAT]
    ut = np.ascontiguousarray(
        np.broadcast_to(ut1[None, :, :], (NG, RANK, FEAT))
    ).reshape(P, FEAT)
    maps = []
    for i in range(N_CORES):
        xs = xb[i * ROWS : (i + 1) * ROWS]
        # xt[c, p, ft, r] = xs[c*CHUNK + r, ft*P + p]
        xt = np.ascontiguousarray(
            xs.reshape(N_CHUNKS, CHUNK, FT, P).transpose(0, 3, 2, 1)
        ).reshape(N_CHUNKS, P, FT * CHUNK)
        maps.append({"xt": xt, "vt": vt, "ut": ut})
    return maps


def kernel(**inputs) -> np.ndarray:
    x = np.asarray(inputs["x"])
    U = inputs["U"]
    S = inputs["S"]
    V = inputs["V"]

    b, sq, feat = x.shape
    x2 = x.reshape(b * sq, feat)

    nc = _get_nc()
    in_maps = make_in_maps(x2, U, S, V)
    res = run_bass_kernel_spmd(nc, in_maps, core_ids=list(range(N_CORES)))
    out = np.concatenate([r["out"] for r in res.results], axis=0)
    return out.astype(np.float32).reshape(b, sq, feat)
